# revision 17
# baseline (speedup 1.0000x reference)
"""DocRE model kernel for 8 Trainium2 NeuronCores.

Data-parallel over the pair grid: core = b*4 + ib owns document b and
i-rows [8*ib, 8*ib+8) of the 32x32 entity-pair grid (256 pairs/core).
All weights are replicated; W_ext (49152x768) is streamed from HBM
through a float32r matmul with the group-bilinear feature tiles
materialized on-chip.
"""

import numpy as np

import concourse.bacc as bacc
import concourse.bass as bass
import concourse.tile as tile
from concourse import mybir
from concourse.bass_utils import run_bass_kernel_spmd
from concourse.masks import make_identity

F32 = mybir.dt.float32
F32R = mybir.dt.float32r
F16 = mybir.dt.float16

B, L, H = 2, 1024, 768
E, M = 32, 4
EMB, BLK, NL = 768, 64, 97
G = EMB // BLK  # 12
LN_EPS = 1e-12

N_CORES = 8
IB = E // (N_CORES // B)     # 8 i-rows per core
NPAIR = IB * E               # 256 pairs per core
PT = NPAIR // 128            # 2 pair-tiles
KT = EMB * BLK // 128        # 384 k-tiles
CT = EMB // 128              # 6 feature chunks
KC = H // 128                # 6 contraction chunks of H
LC = L // 128                # 8 chunks of L
NENT = IB + E + 1            # 41 cols: [my 8 entities | all 32 | cls]


def _build_module():
    nc = bacc.Bacc("TRN2", target_bir_lowering=False, debug=False)

    seq_d = nc.dram_tensor("seq", [L, H], F32R, kind="ExternalInput")
    S_d = nc.dram_tensor("S", [L, NENT], F32R, kind="ExternalInput")
    Wh_d = nc.dram_tensor("Wh", [3 * H, EMB], F16, kind="ExternalInput")
    Wt_d = nc.dram_tensor("Wt", [3 * H, EMB], F16, kind="ExternalInput")
    bh_d = nc.dram_tensor("bh", [128, CT], F32, kind="ExternalInput")
    bt_d = nc.dram_tensor("bt", [128, CT], F32, kind="ExternalInput")
    Wx_d = nc.dram_tensor("Wx", [EMB * BLK, EMB], F16, kind="ExternalInput")
    Ebc_d = nc.dram_tensor("Ebc", [2, 128], F16, kind="ExternalInput")
    bx_d = nc.dram_tensor("bx", [128, EMB], F32, kind="ExternalInput")
    lng_d = nc.dram_tensor("lng", [128, EMB], F32, kind="ExternalInput")
    lnb_d = nc.dram_tensor("lnb", [128, EMB], F32, kind="ExternalInput")
    Wc_d = nc.dram_tensor("Wc", [EMB, NL], F32, kind="ExternalInput")
    out_d = nc.dram_tensor("out", [NPAIR, NL], F32, kind="ExternalOutput")

    with tile.TileContext(nc) as tc:
        with (
            tc.tile_pool(name="persist", bufs=1) as persist,
            tc.tile_pool(name="seqp", bufs=1) as seqp,
            tc.tile_pool(name="whp", bufs=4) as whp,
            tc.tile_pool(name="wxp", bufs=24) as wxp,
            tc.tile_pool(name="blp", bufs=6) as blp,
            tc.tile_pool(name="hsgp", bufs=3) as hsgp,
            tc.tile_pool(name="tmpp", bufs=3) as tmpp,
            tc.tile_pool(name="dramp", bufs=1, space="DRAM") as dramp,
            tc.tile_pool(name="psf", bufs=1, space="PSUM") as psf,
            tc.tile_pool(name="psg", bufs=3, space="PSUM") as psg,
            tc.tile_pool(name="psb", bufs=1, space="PSUM") as psb,
        ):
            ident = persist.tile([128, 128], F32, name="ident")
            make_identity(nc, ident[:])
            E_t = persist.tile([2, 128], F16, name="E_t")
            nc.sync.dma_start(E_t[:], Ebc_d.ap())

            # ---- per-column constants broadcast to all partitions ----
            bx_b = persist.tile([128, EMB], F32, name="bx_b")
            lng_b = persist.tile([128, EMB], F32, name="lng_b")
            lnb_b = persist.tile([128, EMB], F32, name="lnb_b")
            for tile_, src in ((bx_b, bx_d), (lng_b, lng_d), (lnb_b, lnb_d)):
                nc.sync.dma_start(tile_[:], src.ap())

            eps_t = persist.tile([128, 1], F32, name="eps")
            nc.vector.memset(eps_t[:], LN_EPS)

            # per-partition bias chunks bh/bt: [128, CT]
            bh_t = persist.tile([128, CT], F32, name="bh_t")
            bt_t = persist.tile([128, CT], F32, name="bt_t")
            for tile_, src in ((bh_t, bh_d), (bt_t, bt_d)):
                nc.sync.dma_start(tile_[:], src.ap())

            # ---- phase E: entity pooling  ent = S^T @ seq ----
            seq_t = seqp.tile([128, LC, H], F32R, name="seq_t")
            nc.sync.dma_start(seq_t[:], seq_d.ap().rearrange("(c p) h -> p c h", p=128))
            S_t = seqp.tile([128, LC, NENT], F32R, name="S_t")
            nc.sync.dma_start(S_t[:], S_d.ap().rearrange("(c p) n -> p c n", p=128))

            ps_e0 = psg.tile([NENT, 512], F32, name="gen")
            ps_e1 = psg.tile([NENT, 256], F32, name="gen")
            for kc in range(LC):
                nc.tensor.matmul(ps_e0[:], S_t[:, kc, :], seq_t[:, kc, 0:512],
                                 start=(kc == 0), stop=(kc == LC - 1))
                nc.tensor.matmul(ps_e1[:], S_t[:, kc, :], seq_t[:, kc, 512:768],
                                 start=(kc == 0), stop=(kc == LC - 1))
            ent_nat = persist.tile([NENT, H], F32, name="ent_nat")
            nc.scalar.copy(ent_nat[:, 0:512], ps_e0[:])
            nc.scalar.copy(ent_nat[:, 512:768], ps_e1[:])

            # transpose ent -> entT [h, NENT]  (f32r: feeds phase-A matmuls)
            entT = persist.tile([128, KC, NENT], F16, name="entT")
            for kc in range(KC):
                ps_tr = psg.tile([128, NENT], F32, name="gen")
                nc.tensor.transpose(ps_tr[:], ent_nat[:, kc * 128:(kc + 1) * 128],
                                    ident[:NENT, :NENT])
                nc.scalar.copy(entT[:, kc, :], ps_tr[:])

            # ---- phase A: A/B/C projections ----
            # natural layout first: X_nat = ent @ W_block  [41, 768], then
            # PE-transpose into ABCD[ct][:, m, :] ([c,41], m: Ah,Bh,At,Bt).
            ABCD = []
            for ct in range(CT):
                abcd_alloc = persist.tile([128, 4, NENT + 1], F32, name=f"abcd{ct}")
                nc.vector.memset(abcd_alloc[:], 0.0)
                ABCD.append(abcd_alloc)

            def emit_ab_chain(m, w_d, blk):
                ps_n0 = psg.tile([NENT, 512], F32, name="gen")
                ps_n1 = psg.tile([NENT, 256], F32, name="gen")
                for kc in range(KC):
                    w_t = whp.tile([128, EMB], F16, name="w_t")
                    nc.sync.dma_start(
                        w_t[:], w_d.ap()[blk * H + kc * 128: blk * H + (kc + 1) * 128, :])
                    nc.tensor.matmul(ps_n0[:], entT[:, kc, :], w_t[:, 0:512],
                                     start=(kc == 0), stop=(kc == KC - 1))
                    nc.tensor.matmul(ps_n1[:], entT[:, kc, :], w_t[:, 512:768],
                                     start=(kc == 0), stop=(kc == KC - 1))
                x_nat = tmpp.tile([NENT, EMB], F32, name="x_nat")
                nc.scalar.copy(x_nat[:, 0:512], ps_n0[:])
                nc.scalar.copy(x_nat[:, 512:768], ps_n1[:])
                for ct in range(CT):
                    ps_tr = psg.tile([128, NENT], F32, name="gen")
                    nc.tensor.transpose(ps_tr[:], x_nat[:, ct * 128:(ct + 1) * 128],
                                        ident[:NENT, :NENT])
                    nc.scalar.copy(ABCD[ct][:, m, 0:NENT], ps_tr[:])

            def emit_c_chain(m_sel, w_d, bias_t):
                ps_c0 = psg.tile([NENT, 512], F32, name="gen")
                ps_c1 = psg.tile([NENT, 256], F32, name="gen")
                for kc in range(KC):
                    w_t = whp.tile([128, EMB], F16, name="w_t")
                    nc.sync.dma_start(
                        w_t[:], w_d.ap()[2 * H + kc * 128: 2 * H + (kc + 1) * 128, :])
                    nc.tensor.matmul(ps_c0[:1, :], entT[:, kc, IB + E:IB + E + 1],
                                     w_t[:, 0:512],
                                     start=(kc == 0), stop=(kc == KC - 1))
                    nc.tensor.matmul(ps_c1[:1, :], entT[:, kc, IB + E:IB + E + 1],
                                     w_t[:, 512:768],
                                     start=(kc == 0), stop=(kc == KC - 1))
                c_nat = tmpp.tile([1, EMB], F32, name="c_nat")
                nc.scalar.copy(c_nat[:, 0:512], ps_c0[:1, :])
                nc.scalar.copy(c_nat[:, 512:768], ps_c1[:1, :])
                for ct in range(CT):
                    ps_tr = psg.tile([128, NENT], F32, name="gen")
                    nc.tensor.transpose(ps_tr[:, 0:1],
                                        c_nat[:, ct * 128:(ct + 1) * 128],
                                        ident[:1, :1])
                    nc.vector.tensor_tensor(ABCD[ct][:, m_sel, NENT:NENT + 1],
                                            ps_tr[:, 0:1],
                                            bias_t[:, ct:ct + 1],
                                            op=mybir.AluOpType.add)

            emit_c_chain(3, Wt_d, bt_t)
            emit_ab_chain(2, Wt_d, 0)
            emit_ab_chain(3, Wt_d, 1)


            # ---- phase P ts-side: tsdup generated from duplicated ABCD ----
            # col = pt*128 + il*32 + j ; i = 8*ib + pt*4 + il
            hsT = persist.tile([128, CT, 2 * 128], F16, name="hsT")
            tsdup = persist.tile([128, G, 2 * 128], F16, name="tsdup")
            hs_dram = dramp.tile([EMB, 2 * 128], F16, name="hs_dram")
            NE2 = NENT + 1

            def colview(tile_, m, col0, ap_pat):
                return bass.AP(tensor=tile_.tensor,
                               offset=tile_.offset + m * NE2 + col0,
                               ap=[tile_.ap[0]] + ap_pat)

            for ct in range(CT):
                abcd_t = ABCD[ct]
                for half in range(2):
                    g = 2 * ct + half
                    dup_t = tmpp.tile([128, 4, NE2], F32, name="dup")
                    src_ab = abcd_t[half * 64:half * 64 + 64, :, :]
                    nc.scalar.dma_start(dup_t[0:64, :, :], src_ab)
                    nc.scalar.dma_start(dup_t[64:128, :, :], src_ab)
                    tmp2 = tmpp.tile([128, 8, 32], F32, name="tmp")
                    nc.vector.tensor_tensor(
                        tmp2[:], colview(dup_t, 2, IB, [[0, 8], [1, 32]]),
                        colview(dup_t, 3, 0, [[1, 8], [0, 32]]),
                        op=mybir.AluOpType.add)
                    nc.scalar.activation(
                        tsdup[:, g, :].rearrange("p (a b) -> p a b", a=8),
                        tmp2[:], mybir.ActivationFunctionType.Tanh,
                        bias=dup_t[:, 3, NENT:NENT + 1], scale=1.0)

            # ---- head-side projections, then hs generation ----
            emit_c_chain(0, Wh_d, bh_t)
            emit_ab_chain(0, Wh_d, 0)
            emit_ab_chain(1, Wh_d, 1)
            for ct in range(CT):
                abcd_t = ABCD[ct]
                tmp = tmpp.tile([128, 8, 32], F32, name="tmp")
                nc.vector.tensor_tensor(
                    tmp[:], colview(abcd_t, 0, 0, [[1, 8], [0, 32]]),
                    colview(abcd_t, 1, IB, [[0, 8], [1, 32]]),
                    op=mybir.AluOpType.add)
                nc.scalar.activation(
                    hsT[:, ct, :].rearrange("p (a b) -> p a b", a=8),
                    tmp[:], mybir.ActivationFunctionType.Tanh,
                    bias=abcd_t[:, 0, NENT:NENT + 1], scale=1.0)
                nc.scalar.dma_start(hs_dram[ct * 128:(ct + 1) * 128, :],
                                    hsT[:, ct, :])

            # ---- phase M: main contraction over W_ext ----
            ps_feat = [[psf.tile([128, 512], F32, name=f"pf{pt}a"),
                        psf.tile([128, 256], F32, name=f"pf{pt}b")]
                       for pt in range(PT)]
            hsg_t = None
            for kt in range(KT):
                g, t = kt // 32, kt % 32
                ct, half = g // 2, g % 2
                wx_t = wxp.tile([128, EMB], F16, name="wx_t")
                nc.sync.dma_start(wx_t[:], Wx_d.ap()[kt * 128:(kt + 1) * 128, :])

                if t == 0:
                    # stage group g's 64 hs rows into partitions 0-1:
                    # hsg[r, tt, :] = hs row (g*64 + 2*tt + r)
                    hsg_t = hsgp.tile([2, 32, 2 * 128], F16, name="hsg")
                    nc.scalar.dma_start(
                        hsg_t[:],
                        bass.AP(tensor=hs_dram.tensor,
                                offset=hs_dram.offset + g * 64 * 2 * 128,
                                ap=[[2 * 128, 2], [2 * 2 * 128, 32], [1, 2 * 128]]))

                bc_ps = psb.tile([128, 2 * 128], F32, name="bc_ps")
                nc.tensor.matmul(bc_ps[:], E_t[:], hsg_t[:, t, :],
                                 start=True, stop=True)
                bl_t = blp.tile([128, 2 * 128], F16, name="bl_t")
                nc.vector.tensor_tensor(bl_t[:], bc_ps[:], tsdup[:, g, :],
                                        op=mybir.AluOpType.mult)
                for pt in range(PT):
                    lhsT = bl_t[:, pt * 128:(pt + 1) * 128]
                    nc.tensor.matmul(ps_feat[pt][0][:], lhsT, wx_t[:, 0:512],
                                     start=(kt == 0), stop=(kt == KT - 1))
                    nc.tensor.matmul(ps_feat[pt][1][:], lhsT, wx_t[:, 512:768],
                                     start=(kt == 0), stop=(kt == KT - 1))

            # ---- phase L: bias, relu, layernorm, classifier ----
            wc_t = persist.tile([128, CT, NL], F32, name="wc_t")
            nc.sync.dma_start(wc_t[:], Wc_d.ap().rearrange("(c p) n -> p c n", p=128))

            for pt in range(PT):
                feat = persist.tile([128, EMB], F32, name=f"feat{pt}")
                nc.vector.tensor_tensor(feat[:, 0:512], ps_feat[pt][0][:],
                                        bx_b[:, 0:512], op=mybir.AluOpType.add)
                nc.vector.tensor_tensor(feat[:, 512:768], ps_feat[pt][1][:],
                                        bx_b[:, 512:768], op=mybir.AluOpType.add)
                nc.vector.tensor_scalar_max(feat[:], feat[:], 0.0)

                stats = tmpp.tile([128, 3, 6], F32, name="stats")
                f_re = feat.rearrange("p (c f) -> p c f", c=3)
                for c in range(3):
                    nc.vector.bn_stats(stats[:, c, :], f_re[:, c, :])
                mv = tmpp.tile([128, 2], F32, name="mv")
                nc.vector.bn_aggr(mv[:], stats[:])
                sd = tmpp.tile([128, 1], F32, name="sd")
                nc.scalar.activation(sd[:], mv[:, 1:2],
                                     mybir.ActivationFunctionType.Sqrt,
                                     bias=eps_t[:], scale=1.0)
                rstd = tmpp.tile([128, 1], F32, name="rstd")
                nc.vector.reciprocal(rstd[:], sd[:])

                ln = persist.tile([128, EMB], F32, name=f"ln{pt}")
                nc.vector.tensor_scalar(ln[:], feat[:], mv[:, 0:1], rstd[:],
                                        op0=mybir.AluOpType.subtract,
                                        op1=mybir.AluOpType.mult)
                nc.vector.tensor_tensor(ln[:], ln[:], lng_b[:],
                                        op=mybir.AluOpType.mult)
                nc.vector.tensor_tensor(ln[:], ln[:], lnb_b[:],
                                        op=mybir.AluOpType.add)

                lnT = persist.tile([128, CT, 128], F32, name=f"lnT{pt}")
                for ct in range(CT):
                    ps_tr2 = psg.tile([128, 128], F32, name="gen")
                    nc.tensor.transpose(ps_tr2[:], ln[:, ct * 128:(ct + 1) * 128],
                                        ident[:])
                    nc.scalar.copy(lnT[:, ct, :], ps_tr2[:])

                ps_lg = psg.tile([128, NL], F32, name="gen")
                for ct in range(CT):
                    nc.tensor.matmul(ps_lg[:], lnT[:, ct, :], wc_t[:, ct, :],
                                     start=(ct == 0), stop=(ct == CT - 1))
                out_sb = tmpp.tile([128, NL], F32, name="out_sb")
                nc.scalar.copy(out_sb[:], ps_lg[:])
                nc.scalar.dma_start(out_d.ap()[pt * 128:(pt + 1) * 128, :], out_sb[:])

    nc.compile()
    return nc


_NC_CACHE = []


def _get_module():
    if not _NC_CACHE:
        _NC_CACHE.append(_build_module())
    return _NC_CACHE[0]


_EBC = np.zeros((2, 128), np.float16)
_EBC[0, :64] = 1.0
_EBC[1, 64:] = 1.0


def _build_inputs(seq, starts, ends, mention_mask, W_head, b_head, W_tail, b_tail,
                  W_ext, b_ext, ln_g, ln_b, W_cls):
    seq = np.asarray(seq, np.float32)
    starts = np.asarray(starts, np.int64)
    ends = np.asarray(ends, np.int64)
    mask = np.asarray(mention_mask, np.float32)

    # per-document entity selection matrix: ent = Sb^T @ seq[b]
    S_b = np.zeros((B, L, E), np.float32)
    denom = np.maximum(mask.sum(axis=2), 1.0)          # [B, E]
    w = mask * 0.5 / denom[:, :, None]                 # [B, E, M]
    for b in range(B):
        for e in range(E):
            np.add.at(S_b[b, :, e], starts[b, e] + 1, w[b, e])
            np.add.at(S_b[b, :, e], ends[b, e], w[b, e])

    cls_col = np.zeros((L, 1), np.float32)
    cls_col[0, 0] = 1.0

    shared = {
        "Wh": np.ascontiguousarray(np.asarray(W_head, np.float32).astype(np.float16)),
        "Wt": np.ascontiguousarray(np.asarray(W_tail, np.float32).astype(np.float16)),
        "bh": np.ascontiguousarray(np.asarray(b_head, np.float32).reshape(CT, 128).T),
        "bt": np.ascontiguousarray(np.asarray(b_tail, np.float32).reshape(CT, 128).T),
        "Wx": np.ascontiguousarray(np.asarray(W_ext).astype(np.float16)),
        "Ebc": _EBC,
        "bx": np.ascontiguousarray(np.broadcast_to(np.asarray(b_ext, np.float32), (128, EMB))),
        "lng": np.ascontiguousarray(np.broadcast_to(np.asarray(ln_g, np.float32), (128, EMB))),
        "lnb": np.ascontiguousarray(np.broadcast_to(np.asarray(ln_b, np.float32), (128, EMB))),
        "Wc": np.ascontiguousarray(W_cls, dtype=np.float32),
    }
    in_maps = []
    for core in range(N_CORES):
        b, ib = core // 4, core % 4
        S_core = np.concatenate(
            [S_b[b][:, ib * IB:(ib + 1) * IB], S_b[b], cls_col], axis=1)
        in_maps.append({
            "seq": np.ascontiguousarray(seq[b]),
            "S": np.ascontiguousarray(S_core),
            **shared,
        })
    return in_maps


def kernel(**inputs) -> np.ndarray:
    nc = _get_module()
    in_maps = _build_inputs(**inputs)
    res = run_bass_kernel_spmd(nc, in_maps, core_ids=list(range(N_CORES)))
    outs = np.stack([res.results[c]["out"] for c in range(N_CORES)])  # [8,256,97]
    return outs.reshape(B, 4, IB, E, NL).reshape(B, E, E, NL)


# revision 18
# speedup vs baseline: 1.8080x; 1.8080x over previous
"""DocRE model kernel for 8 Trainium2 NeuronCores.

Data-parallel over the pair grid: core = b*4 + ib owns document b and
i-rows [8*ib, 8*ib+8) of the 32x32 entity-pair grid (256 pairs/core).
All weights are replicated; W_ext (49152x768) is streamed from HBM
through a float32r matmul with the group-bilinear feature tiles
materialized on-chip.
"""

import numpy as np

import concourse.bacc as bacc
import concourse.bass as bass
import concourse.tile as tile
from concourse import mybir
from concourse.bass_utils import run_bass_kernel_spmd
from concourse.masks import make_identity

F32 = mybir.dt.float32
F32R = mybir.dt.float32r
F16 = mybir.dt.float16

B, L, H = 2, 1024, 768
E, M = 32, 4
EMB, BLK, NL = 768, 64, 97
G = EMB // BLK  # 12
LN_EPS = 1e-12

N_CORES = 8
IB = E // (N_CORES // B)     # 8 i-rows per core
NPAIR = IB * E               # 256 pairs per core
PT = NPAIR // 128            # 2 pair-tiles
KT = EMB * BLK // 128        # 384 k-tiles
CT = EMB // 128              # 6 feature chunks
KC = H // 128                # 6 contraction chunks of H
LC = L // 128                # 8 chunks of L
NENT = IB + E + 1            # 41 cols: [my 8 entities | all 32 | cls]


def _build_module():
    nc = bacc.Bacc("TRN2", target_bir_lowering=False, debug=False)

    seq_d = nc.dram_tensor("seq", [L, H], F32R, kind="ExternalInput")
    S_d = nc.dram_tensor("S", [L, NENT], F32R, kind="ExternalInput")
    Wh_d = nc.dram_tensor("Wh", [3 * H, EMB], F16, kind="ExternalInput")
    Wt_d = nc.dram_tensor("Wt", [3 * H, EMB], F16, kind="ExternalInput")
    bh_d = nc.dram_tensor("bh", [128, CT], F32, kind="ExternalInput")
    bt_d = nc.dram_tensor("bt", [128, CT], F32, kind="ExternalInput")
    Wx_d = nc.dram_tensor("Wx", [EMB * BLK, EMB], F16, kind="ExternalInput")
    Ebc_d = nc.dram_tensor("Ebc", [2, 128], F16, kind="ExternalInput")
    bx_d = nc.dram_tensor("bx", [128, EMB], F32, kind="ExternalInput")
    lng_d = nc.dram_tensor("lng", [128, EMB], F32, kind="ExternalInput")
    lnb_d = nc.dram_tensor("lnb", [128, EMB], F32, kind="ExternalInput")
    Wc_d = nc.dram_tensor("Wc", [EMB, NL], F32, kind="ExternalInput")
    out_d = nc.dram_tensor("out", [NPAIR, NL], F32, kind="ExternalOutput")

    with tile.TileContext(nc) as tc:
        with (
            tc.tile_pool(name="persist", bufs=1) as persist,
            tc.tile_pool(name="seqp", bufs=1) as seqp,
            tc.tile_pool(name="whp", bufs=4) as whp,
            tc.tile_pool(name="wxp", bufs=24) as wxp,
            tc.tile_pool(name="blp", bufs=6) as blp,
            tc.tile_pool(name="hsgp", bufs=3) as hsgp,
            tc.tile_pool(name="tmpp", bufs=3) as tmpp,
            tc.tile_pool(name="dramp", bufs=1, space="DRAM") as dramp,
            tc.tile_pool(name="psf", bufs=1, space="PSUM") as psf,
            tc.tile_pool(name="psg", bufs=2, space="PSUM") as psg,
            tc.tile_pool(name="psb", bufs=2, space="PSUM") as psb,
        ):
            ident = persist.tile([128, 128], F32, name="ident")
            make_identity(nc, ident[:])
            E_t = persist.tile([2, 128], F16, name="E_t")
            nc.sync.dma_start(E_t[:], Ebc_d.ap())

            # ---- per-column constants broadcast to all partitions ----
            bx_b = persist.tile([128, EMB], F32, name="bx_b")
            lng_b = persist.tile([128, EMB], F32, name="lng_b")
            lnb_b = persist.tile([128, EMB], F32, name="lnb_b")
            for tile_, src in ((bx_b, bx_d), (lng_b, lng_d), (lnb_b, lnb_d)):
                nc.sync.dma_start(tile_[:], src.ap())

            eps_t = persist.tile([128, 1], F32, name="eps")
            nc.vector.memset(eps_t[:], LN_EPS)

            # per-partition bias chunks bh/bt: [128, CT]
            bh_t = persist.tile([128, CT], F32, name="bh_t")
            bt_t = persist.tile([128, CT], F32, name="bt_t")
            for tile_, src in ((bh_t, bh_d), (bt_t, bt_d)):
                nc.sync.dma_start(tile_[:], src.ap())

            # ---- phase E: entity pooling  ent = S^T @ seq ----
            seq_t = seqp.tile([128, LC, H], F32R, name="seq_t")
            nc.sync.dma_start(seq_t[:], seq_d.ap().rearrange("(c p) h -> p c h", p=128))
            S_t = seqp.tile([128, LC, NENT], F32R, name="S_t")
            nc.sync.dma_start(S_t[:], S_d.ap().rearrange("(c p) n -> p c n", p=128))

            ps_e0 = psg.tile([NENT, 512], F32, name="gen")
            ps_e1 = psg.tile([NENT, 256], F32, name="gen")
            for kc in range(LC):
                nc.tensor.matmul(ps_e0[:], S_t[:, kc, :], seq_t[:, kc, 0:512],
                                 start=(kc == 0), stop=(kc == LC - 1))
                nc.tensor.matmul(ps_e1[:], S_t[:, kc, :], seq_t[:, kc, 512:768],
                                 start=(kc == 0), stop=(kc == LC - 1))
            ent_nat = persist.tile([NENT, H], F32, name="ent_nat")
            nc.scalar.copy(ent_nat[:, 0:512], ps_e0[:])
            nc.scalar.copy(ent_nat[:, 512:768], ps_e1[:])

            # transpose ent -> entT [h, NENT]  (f32r: feeds phase-A matmuls)
            entT = persist.tile([128, KC, NENT], F16, name="entT")
            for kc in range(KC):
                ps_tr = psg.tile([128, NENT], F32, name="gen")
                nc.tensor.transpose(ps_tr[:], ent_nat[:, kc * 128:(kc + 1) * 128],
                                    ident[:NENT, :NENT])
                nc.scalar.copy(entT[:, kc, :], ps_tr[:])

            # ---- phase A: A/B/C projections ----
            # natural layout first: X_nat = ent @ W_block  [41, 768], then
            # PE-transpose into ABCD[ct][:, m, :] ([c,41], m: Ah,Bh,At,Bt).
            ABCD = []
            for ct in range(CT):
                abcd_alloc = persist.tile([128, 4, NENT + 1], F32, name=f"abcd{ct}")
                nc.vector.memset(abcd_alloc[:], 0.0)
                ABCD.append(abcd_alloc)

            def emit_ab_chain(m, w_d, blk):
                ps_n0 = psg.tile([NENT, 512], F32, name="gen")
                ps_n1 = psg.tile([NENT, 256], F32, name="gen")
                for kc in range(KC):
                    w_t = whp.tile([128, EMB], F16, name="w_t")
                    nc.sync.dma_start(
                        w_t[:], w_d.ap()[blk * H + kc * 128: blk * H + (kc + 1) * 128, :])
                    nc.tensor.matmul(ps_n0[:], entT[:, kc, :], w_t[:, 0:512],
                                     start=(kc == 0), stop=(kc == KC - 1))
                    nc.tensor.matmul(ps_n1[:], entT[:, kc, :], w_t[:, 512:768],
                                     start=(kc == 0), stop=(kc == KC - 1))
                x_nat = tmpp.tile([NENT, EMB], F32, name="x_nat")
                nc.scalar.copy(x_nat[:, 0:512], ps_n0[:])
                nc.scalar.copy(x_nat[:, 512:768], ps_n1[:])
                for ct in range(CT):
                    ps_tr = psg.tile([128, NENT], F32, name="gen")
                    nc.tensor.transpose(ps_tr[:], x_nat[:, ct * 128:(ct + 1) * 128],
                                        ident[:NENT, :NENT])
                    nc.scalar.copy(ABCD[ct][:, m, 0:NENT], ps_tr[:])

            def emit_c_chain(m_sel, w_d, bias_t):
                ps_c0 = psg.tile([NENT, 512], F32, name="gen")
                ps_c1 = psg.tile([NENT, 256], F32, name="gen")
                for kc in range(KC):
                    w_t = whp.tile([128, EMB], F16, name="w_t")
                    nc.sync.dma_start(
                        w_t[:], w_d.ap()[2 * H + kc * 128: 2 * H + (kc + 1) * 128, :])
                    nc.tensor.matmul(ps_c0[:1, :], entT[:, kc, IB + E:IB + E + 1],
                                     w_t[:, 0:512],
                                     start=(kc == 0), stop=(kc == KC - 1))
                    nc.tensor.matmul(ps_c1[:1, :], entT[:, kc, IB + E:IB + E + 1],
                                     w_t[:, 512:768],
                                     start=(kc == 0), stop=(kc == KC - 1))
                c_nat = tmpp.tile([1, EMB], F32, name="c_nat")
                nc.scalar.copy(c_nat[:, 0:512], ps_c0[:1, :])
                nc.scalar.copy(c_nat[:, 512:768], ps_c1[:1, :])
                for ct in range(CT):
                    ps_tr = psg.tile([128, NENT], F32, name="gen")
                    nc.tensor.transpose(ps_tr[:, 0:1],
                                        c_nat[:, ct * 128:(ct + 1) * 128],
                                        ident[:1, :1])
                    nc.vector.tensor_tensor(ABCD[ct][:, m_sel, NENT:NENT + 1],
                                            ps_tr[:, 0:1],
                                            bias_t[:, ct:ct + 1],
                                            op=mybir.AluOpType.add)

            emit_c_chain(3, Wt_d, bt_t)
            emit_ab_chain(2, Wt_d, 0)
            emit_ab_chain(3, Wt_d, 1)


            # ---- phase P ts-side: tsdup generated from duplicated ABCD ----
            # col = pt*128 + il*32 + j ; i = 8*ib + pt*4 + il
            hsT = persist.tile([128, CT, 2 * 128], F16, name="hsT")
            tsdup = persist.tile([128, G, 2 * 128], F16, name="tsdup")
            hs_dram = dramp.tile([EMB, 2 * 128], F16, name="hs_dram")
            NE2 = NENT + 1

            def colview(tile_, m, col0, ap_pat):
                return bass.AP(tensor=tile_.tensor,
                               offset=tile_.offset + m * NE2 + col0,
                               ap=[tile_.ap[0]] + ap_pat)

            for ct in range(CT):
                abcd_t = ABCD[ct]
                for half in range(2):
                    g = 2 * ct + half
                    dup_t = tmpp.tile([128, 4, NE2], F32, name="dup")
                    src_ab = abcd_t[half * 64:half * 64 + 64, :, :]
                    nc.scalar.dma_start(dup_t[0:64, :, :], src_ab)
                    nc.scalar.dma_start(dup_t[64:128, :, :], src_ab)
                    tmp2 = tmpp.tile([128, 8, 32], F32, name="tmp")
                    nc.vector.tensor_tensor(
                        tmp2[:], colview(dup_t, 2, IB, [[0, 8], [1, 32]]),
                        colview(dup_t, 3, 0, [[1, 8], [0, 32]]),
                        op=mybir.AluOpType.add)
                    nc.scalar.activation(
                        tsdup[:, g, :].rearrange("p (a b) -> p a b", a=8),
                        tmp2[:], mybir.ActivationFunctionType.Tanh,
                        bias=dup_t[:, 3, NENT:NENT + 1], scale=1.0)

            # ---- head-side projections, then hs generation ----
            emit_c_chain(0, Wh_d, bh_t)
            emit_ab_chain(0, Wh_d, 0)
            emit_ab_chain(1, Wh_d, 1)
            for ct in range(CT):
                abcd_t = ABCD[ct]
                tmp = tmpp.tile([128, 8, 32], F32, name="tmp")
                nc.vector.tensor_tensor(
                    tmp[:], colview(abcd_t, 0, 0, [[1, 8], [0, 32]]),
                    colview(abcd_t, 1, IB, [[0, 8], [1, 32]]),
                    op=mybir.AluOpType.add)
                nc.scalar.activation(
                    hsT[:, ct, :].rearrange("p (a b) -> p a b", a=8),
                    tmp[:], mybir.ActivationFunctionType.Tanh,
                    bias=abcd_t[:, 0, NENT:NENT + 1], scale=1.0)
                nc.scalar.dma_start(hs_dram[ct * 128:(ct + 1) * 128, :],
                                    hsT[:, ct, :])

            # ---- phase M: main contraction over W_ext ----
            ps_feat = [[psf.tile([128, 512], F32, name=f"pf{pt}a"),
                        psf.tile([128, 256], F32, name=f"pf{pt}b")]
                       for pt in range(PT)]
            hsg_t = None
            for kt in range(KT):
                g, t = kt // 32, kt % 32
                ct, half = g // 2, g % 2
                wx_t = wxp.tile([128, EMB], F16, name="wx_t")
                nc.sync.dma_start(wx_t[:], Wx_d.ap()[kt * 128:(kt + 1) * 128, :])

                if t == 0:
                    # stage group g's 64 hs rows into partitions 0-1:
                    # hsg[r, tt, :] = hs row (g*64 + 2*tt + r)
                    hsg_t = hsgp.tile([2, 32, 2 * 128], F16, name="hsg")
                    nc.scalar.dma_start(
                        hsg_t[:],
                        bass.AP(tensor=hs_dram.tensor,
                                offset=hs_dram.offset + g * 64 * 2 * 128,
                                ap=[[2 * 128, 2], [2 * 2 * 128, 32], [1, 2 * 128]]))

                bc_ps = psb.tile([128, 2 * 128], F32, name="bc_ps")
                nc.tensor.matmul(bc_ps[:], E_t[:], hsg_t[:, t, :],
                                 start=True, stop=True)
                bl_t = blp.tile([128, 2 * 128], F16, name="bl_t")
                nc.vector.tensor_tensor(bl_t[:], bc_ps[:], tsdup[:, g, :],
                                        op=mybir.AluOpType.mult)
                for pt in range(PT):
                    lhsT = bl_t[:, pt * 128:(pt + 1) * 128]
                    nc.tensor.matmul(ps_feat[pt][0][:], lhsT, wx_t[:, 0:512],
                                     start=(kt == 0), stop=(kt == KT - 1))
                    nc.tensor.matmul(ps_feat[pt][1][:], lhsT, wx_t[:, 512:768],
                                     start=(kt == 0), stop=(kt == KT - 1))

            # ---- phase L: bias, relu, layernorm, classifier ----
            wc_t = persist.tile([128, CT, NL], F32, name="wc_t")
            nc.sync.dma_start(wc_t[:], Wc_d.ap().rearrange("(c p) n -> p c n", p=128))

            for pt in range(PT):
                feat = persist.tile([128, EMB], F32, name=f"feat{pt}")
                nc.vector.tensor_tensor(feat[:, 0:512], ps_feat[pt][0][:],
                                        bx_b[:, 0:512], op=mybir.AluOpType.add)
                nc.vector.tensor_tensor(feat[:, 512:768], ps_feat[pt][1][:],
                                        bx_b[:, 512:768], op=mybir.AluOpType.add)
                nc.vector.tensor_scalar_max(feat[:], feat[:], 0.0)

                stats = tmpp.tile([128, 3, 6], F32, name="stats")
                f_re = feat.rearrange("p (c f) -> p c f", c=3)
                for c in range(3):
                    nc.vector.bn_stats(stats[:, c, :], f_re[:, c, :])
                mv = tmpp.tile([128, 2], F32, name="mv")
                nc.vector.bn_aggr(mv[:], stats[:])
                sd = tmpp.tile([128, 1], F32, name="sd")
                nc.scalar.activation(sd[:], mv[:, 1:2],
                                     mybir.ActivationFunctionType.Sqrt,
                                     bias=eps_t[:], scale=1.0)
                rstd = tmpp.tile([128, 1], F32, name="rstd")
                nc.vector.reciprocal(rstd[:], sd[:])

                ln = persist.tile([128, EMB], F32, name=f"ln{pt}")
                nc.vector.tensor_scalar(ln[:], feat[:], mv[:, 0:1], rstd[:],
                                        op0=mybir.AluOpType.subtract,
                                        op1=mybir.AluOpType.mult)
                nc.vector.tensor_tensor(ln[:], ln[:], lng_b[:],
                                        op=mybir.AluOpType.mult)
                nc.vector.tensor_tensor(ln[:], ln[:], lnb_b[:],
                                        op=mybir.AluOpType.add)

                lnT = persist.tile([128, CT, 128], F32, name=f"lnT{pt}")
                for ct in range(CT):
                    ps_tr2 = psg.tile([128, 128], F32, name="gen")
                    nc.tensor.transpose(ps_tr2[:], ln[:, ct * 128:(ct + 1) * 128],
                                        ident[:])
                    nc.scalar.copy(lnT[:, ct, :], ps_tr2[:])

                ps_lg = psg.tile([128, NL], F32, name="gen")
                for ct in range(CT):
                    nc.tensor.matmul(ps_lg[:], lnT[:, ct, :], wc_t[:, ct, :],
                                     start=(ct == 0), stop=(ct == CT - 1))
                out_sb = tmpp.tile([128, NL], F32, name="out_sb")
                nc.scalar.copy(out_sb[:], ps_lg[:])
                nc.scalar.dma_start(out_d.ap()[pt * 128:(pt + 1) * 128, :], out_sb[:])

    nc.compile()
    return nc


_NC_CACHE = []


def _get_module():
    if not _NC_CACHE:
        _NC_CACHE.append(_build_module())
    return _NC_CACHE[0]


_EBC = np.zeros((2, 128), np.float16)
_EBC[0, :64] = 1.0
_EBC[1, 64:] = 1.0


def _build_inputs(seq, starts, ends, mention_mask, W_head, b_head, W_tail, b_tail,
                  W_ext, b_ext, ln_g, ln_b, W_cls):
    seq = np.asarray(seq, np.float32)
    starts = np.asarray(starts, np.int64)
    ends = np.asarray(ends, np.int64)
    mask = np.asarray(mention_mask, np.float32)

    # per-document entity selection matrix: ent = Sb^T @ seq[b]
    S_b = np.zeros((B, L, E), np.float32)
    denom = np.maximum(mask.sum(axis=2), 1.0)          # [B, E]
    w = mask * 0.5 / denom[:, :, None]                 # [B, E, M]
    for b in range(B):
        for e in range(E):
            np.add.at(S_b[b, :, e], starts[b, e] + 1, w[b, e])
            np.add.at(S_b[b, :, e], ends[b, e], w[b, e])

    cls_col = np.zeros((L, 1), np.float32)
    cls_col[0, 0] = 1.0

    shared = {
        "Wh": np.ascontiguousarray(np.asarray(W_head, np.float32).astype(np.float16)),
        "Wt": np.ascontiguousarray(np.asarray(W_tail, np.float32).astype(np.float16)),
        "bh": np.ascontiguousarray(np.asarray(b_head, np.float32).reshape(CT, 128).T),
        "bt": np.ascontiguousarray(np.asarray(b_tail, np.float32).reshape(CT, 128).T),
        "Wx": np.ascontiguousarray(np.asarray(W_ext).astype(np.float16)),
        "Ebc": _EBC,
        "bx": np.ascontiguousarray(np.broadcast_to(np.asarray(b_ext, np.float32), (128, EMB))),
        "lng": np.ascontiguousarray(np.broadcast_to(np.asarray(ln_g, np.float32), (128, EMB))),
        "lnb": np.ascontiguousarray(np.broadcast_to(np.asarray(ln_b, np.float32), (128, EMB))),
        "Wc": np.ascontiguousarray(W_cls, dtype=np.float32),
    }
    in_maps = []
    for core in range(N_CORES):
        b, ib = core // 4, core % 4
        S_core = np.concatenate(
            [S_b[b][:, ib * IB:(ib + 1) * IB], S_b[b], cls_col], axis=1)
        in_maps.append({
            "seq": np.ascontiguousarray(seq[b]),
            "S": np.ascontiguousarray(S_core),
            **shared,
        })
    return in_maps


def kernel(**inputs) -> np.ndarray:
    nc = _get_module()
    in_maps = _build_inputs(**inputs)
    res = run_bass_kernel_spmd(nc, in_maps, core_ids=list(range(N_CORES)))
    outs = np.stack([res.results[c]["out"] for c in range(N_CORES)])  # [8,256,97]
    return outs.reshape(B, 4, IB, E, NL).reshape(B, E, E, NL)
